# revision 44
# baseline (speedup 1.0000x reference)
"""Cross-attention Trainium2 kernel (nn_CrossAttention_7627861918199).

Full-input contract: kernel(**inputs) takes the unsharded numpy inputs and
returns the full [B, NQ, D] float32 output.

Sharding: 8 cores = (batch b, query-half qh); core c handles batch c//2,
queries [(c%2)*512, (c%2)*512+512).  No collectives.  Each core runs a fused
attention pipeline:
  qT = Wq @ xT            (inner on partitions)
  kT = Wk @ ctxT          (kdim on partitions)
  v  = ctx @ Wv^T         (nkv on partitions, + ones column for softmax sums)
  per head pair: S^T chunks [128kv, nq] = k @ qT (PE row-tiled pair)
            -> exp (ACT, scale=1/8) -> P^T bf16
            -> O_aug[q 128, 65] += P^T(stationary) x v_aug(moving)  (flipped
               AV: N=65 so the PE pair of heads costs half of the M=65 form)
  normalize per q-slice: o = O[:, :64] * recip(O[:, 64])  (per-partition
  tensor_scalar -- no cross-partition broadcast needed)
  transpose o back to [d, q] on the PE (identity matmul) for the out-proj
  y = O @ Wo^T + bo
All matmul inputs bf16 (fp32 accumulate); host pre-transposes x/context/
weights so the kernel needs no on-device input transposes.
"""

import numpy as np
import ml_dtypes

# Accumulate out-projection contributions per head pair into SBUF during the
# attention loop (leaves only one matmul per q-chunk after the last pair).
PARTIAL_M5 = True

HEADS = 8
DIM_HEAD = 64
D = 512          # QUERY_DIM == inner dim
B, NQ, NKV = 4, 1024, 4096
N_CORES = 8
NQ_SH = B * NQ // N_CORES   # 512 queries per core
P = 128
DC = D // P                  # 4 contraction chunks of 128
NCHUNK = NKV // P            # 32 kv chunks of 128
NT = NKV // 512              # 8 n-tiles for kT
NQS = NQ_SH // P             # 4 query slices of 128
BF16 = ml_dtypes.bfloat16

# AV matmuls lag scores/exp by SKEW chunks so the in-order PE stream
# doesn't block on the scalar-engine exp latency.  Must be >= NQS so the
# previous head pair's 4 drain slices (emitted at j=1..NQS) all precede
# this pair's first AV write to the shared (bufs=1) oq accumulators.
SKEW = 4

_PROGRAMS = {}


def _build(need_mask: bool, num_devices: int = N_CORES):
    import concourse.mybir as mybir
    import concourse.tile as tile
    from concourse import bacc
    from concourse import masks

    dt = mybir.dt
    f32, bf = dt.float32, dt.bfloat16
    nq = NQ_SH

    nc = bacc.Bacc("TRN2", target_bir_lowering=False, debug=False,
                   num_devices=num_devices)

    xT = nc.dram_tensor("xT", [D, nq], bf, kind="ExternalInput").ap()
    ctxT = nc.dram_tensor("ctxT", [D, NKV], bf, kind="ExternalInput").ap()
    wqT = nc.dram_tensor("wqT", [D, D], bf, kind="ExternalInput").ap()
    wkT = nc.dram_tensor("wkT", [D, D], bf, kind="ExternalInput").ap()
    wvT = nc.dram_tensor("wvT", [D, D], bf, kind="ExternalInput").ap()
    woT = nc.dram_tensor("woT", [D, D], bf, kind="ExternalInput").ap()
    bo = nc.dram_tensor("bo", [1, D], f32, kind="ExternalInput").ap()
    if need_mask:
        maskb = nc.dram_tensor("maskb", [P, NCHUNK], f32,
                               kind="ExternalInput").ap()
    y = nc.dram_tensor("y", [nq, D], f32, kind="ExternalOutput").ap()

    Exp = mybir.ActivationFunctionType.Exp

    with tile.TileContext(nc) as tc:
        with tc.tile_pool(name="big", bufs=1) as big, \
             tc.tile_pool(name="work", bufs=4) as work, \
             tc.tile_pool(name="pTp", bufs=6) as pTp, \
             tc.tile_pool(name="proj_ps", bufs=2, space="PSUM") as proj_ps, \
             tc.tile_pool(name="score_ps", bufs=2, space="PSUM") as score_ps, \
             tc.tile_pool(name="oq_ps", bufs=1, space="PSUM") as oq_ps:

            ctx_sb = big.tile([P, DC, NKV], bf, name="ctx_sb")
            x_sb = big.tile([P, DC, nq], bf, name="x_sb")
            wq_sb = big.tile([P, DC, D], bf, name="wq_sb")
            wk_sb = big.tile([P, DC, D], bf, name="wk_sb")
            wv_sb = big.tile([P, DC, D], bf, name="wv_sb")
            wo_sb = big.tile([P, DC, D], bf, name="wo_sb")
            bo_bc = big.tile([P, D], f32, name="bo_bc")
            q_sb = big.tile([P, DC, nq], bf, name="q_sb")
            k_sb = big.tile([P, DC, NKV], bf, name="k_sb")
            v_sb = big.tile([P, NCHUNK, HEADS, DIM_HEAD + 1], bf, name="v_sb")
            o_sb = big.tile([P, DC, nq], bf, name="o_sb")
            y_acc = big.tile([P, NQS, D], f32, name="y_acc")
            ident = big.tile([P, P], f32, name="ident")
            if need_mask:
                mb_sb = big.tile([P, NCHUNK], f32, name="mb_sb")

            # ---- input DMAs: ctx chunk 0 + wk first (they gate the first
            # scores via K-proj), then x/wq, then the remaining ctx chunks.
            xTr = xT.rearrange("(c p) n -> p c n", p=P)
            wqr = wqT.rearrange("(c p) n -> p c n", p=P)
            wkr = wkT.rearrange("(c p) n -> p c n", p=P)
            wvr = wvT.rearrange("(c p) n -> p c n", p=P)
            wor = woT.rearrange("(c p) n -> p c n", p=P)
            ctxr = ctxT.rearrange("(c p) n -> p c n", p=P)
            # The first exp is gated by: x (full), wq/wk cols 0:128 (only
            # ic=0 slices are touched before ~17us) and ctx cols 0:512.
            # Those pieces go first as SMALL transfers; everything else
            # streams behind them (chip HBM is saturated by all 8 cores
            # loading at once, so order and size matter).
            nc.sync.dma_start(x_sb[:], xTr)
            nc.gpsimd.dma_start(wq_sb[:, :, 0:P], wqr[:, :, 0:P])
            nc.sync.dma_start(wk_sb[:, :, 0:P], wkr[:, :, 0:P])
            nc.gpsimd.dma_start(ctx_sb[:, :, 0:P], ctxr[:, :, 0:P])
            nc.sync.dma_start(ctx_sb[:, :, P:512], ctxr[:, :, P:512])
            nc.gpsimd.dma_start(wq_sb[:, :, P:512], wqr[:, :, P:512])
            nc.sync.dma_start(wk_sb[:, :, P:512], wkr[:, :, P:512])
            nc.gpsimd.dma_start(wv_sb[:], wvr)
            for nt in range(1, NT):
                eng = nc.sync if nt % 2 == 1 else nc.gpsimd
                eng.dma_start(ctx_sb[:, :, nt * 512:(nt + 1) * 512],
                              ctxr[:, :, nt * 512:(nt + 1) * 512])
            nc.sync.dma_start(wo_sb[:], wor)
            nc.sync.dma_start(bo_bc[:], bo.to_broadcast([P, D]))
            if need_mask:
                nc.sync.dma_start(mb_sb[:], maskb[:])
            nc.vector.memset(v_sb[:, :, :, DIM_HEAD], 1.0)
            masks.make_identity(nc, ident[:])

            # ---- projection emitters; _first/_rest split a 4-matmul
            # contraction burst around the score pair so the scalar engine's
            # exp stream is never starved by a long PE burst.
            def m0_first(ic):
                ps = proj_ps.tile([P, 512], f32, name="ps_proj", tag="proj")
                for kc in (0, 1):
                    nc.tensor.matmul(
                        ps[:, :nq], wq_sb[:, kc, ic * P:(ic + 1) * P],
                        x_sb[:, kc, :], start=(kc == 0), stop=False)
                return ps

            def m0_rest(ic, ps):
                for kc in (2, 3):
                    nc.tensor.matmul(
                        ps[:, :nq], wq_sb[:, kc, ic * P:(ic + 1) * P],
                        x_sb[:, kc, :], start=False, stop=(kc == DC - 1))
                nc.vector.tensor_copy(out=q_sb[:, ic, :], in_=ps[:, :nq])

            def emit_m0(ic):
                m0_rest(ic, m0_first(ic))

            def m1_first(ic, nt):
                ps = proj_ps.tile([P, 512], f32, name="ps_proj", tag="proj")
                for kc in (0, 1):
                    nc.tensor.matmul(
                        ps, wk_sb[:, kc, ic * P:(ic + 1) * P],
                        ctx_sb[:, kc, nt * 512:(nt + 1) * 512],
                        start=(kc == 0), stop=False)
                return ps

            def m1_rest(ic, nt, ps):
                for kc in (2, 3):
                    nc.tensor.matmul(
                        ps, wk_sb[:, kc, ic * P:(ic + 1) * P],
                        ctx_sb[:, kc, nt * 512:(nt + 1) * 512],
                        start=False, stop=(kc == DC - 1))
                nc.vector.tensor_copy(
                    out=k_sb[:, ic, nt * 512:(nt + 1) * 512], in_=ps)

            def emit_m1(ic, nt):
                m1_rest(ic, nt, m1_first(ic, nt))

            def emit_v(j, hp):
                # v columns for head pair hp only: [128kv, 128]
                ps = proj_ps.tile([P, 512], f32, name="ps_proj", tag="proj")
                for kc in range(DC):
                    nc.tensor.matmul(
                        ps[:, 0:P], ctx_sb[:, kc, j * P:(j + 1) * P],
                        wv_sb[:, kc, hp * P:(hp + 1) * P],
                        start=(kc == 0), stop=(kc == DC - 1))
                nc.vector.tensor_copy(
                    out=v_sb[:, j, 2 * hp:2 * hp + 2, 0:DIM_HEAD],
                    in_=ps[:, 0:P].rearrange("p (h d) -> p h d", h=2))

            # Out-projection is accumulated per-ic into y_acc (SBUF) as soon
            # as each head pair's o_sb slices land, so only ic=3 plus the
            # final add + store remain after the last head pair.
            def emit_m5_partial(ic, qc):
                ps = proj_ps.tile([P, 512], f32, name="ps_proj", tag="proj")
                nc.tensor.matmul(ps, o_sb[:, ic, qc * P:(qc + 1) * P],
                                 wo_sb[:, ic, :], start=True, stop=True)
                if ic == 0:
                    nc.vector.tensor_add(y_acc[:, qc, :], ps, bo_bc)
                else:
                    nc.vector.tensor_add(y_acc[:, qc, :], ps, y_acc[:, qc, :])

            def emit_m5_final(qc):
                ps = proj_ps.tile([P, 512], f32, name="ps_proj", tag="proj")
                if PARTIAL_M5:
                    nc.tensor.matmul(ps,
                                     o_sb[:, DC - 1, qc * P:(qc + 1) * P],
                                     wo_sb[:, DC - 1, :],
                                     start=True, stop=True)
                    y_sb = work.tile([P, D], f32, name="y_sb", tag="y_sb")
                    nc.vector.tensor_add(y_sb, ps, y_acc[:, qc, :])
                else:
                    for ic2 in range(DC):
                        nc.tensor.matmul(ps,
                                         o_sb[:, ic2, qc * P:(qc + 1) * P],
                                         wo_sb[:, ic2, :],
                                         start=(ic2 == 0),
                                         stop=(ic2 == DC - 1))
                    y_sb = work.tile([P, D], f32, name="y_sb", tag="y_sb")
                    nc.vector.tensor_add(y_sb, ps, bo_bc)
                # sync + scalar queues are idle at the tail (gpsimd still
                # drains earlier DMAs); splitting queues overlaps transfers
                eng = nc.sync if qc % 2 == 0 else nc.scalar
                eng.dma_start(y[qc * P:(qc + 1) * P, :], y_sb)

            def emit_drain(hp, oqs, s, do_m5):
                # softmax-normalize + transpose one q-slice of a head pair,
                # writing o_sb[d, ic=hp, s*128:(s+1)*128].  Both parities'
                # 64 d-columns sit side by side in o_q, so ONE [128,128] f32
                # transpose (psum partition 0, as walrus requires) yields the
                # o_sb row layout directly -- no bitcast views.
                ic = hp
                o_q = work.tile([P, P], f32, name="o_q", tag="o_q")
                for par in range(2):
                    oq = oqs[par]
                    rec = work.tile([P, 1], f32, name="rec", tag="rec")
                    nc.vector.reciprocal(rec[:, 0:1],
                                         oq[:, s, DIM_HEAD:DIM_HEAD + 1])
                    po = par * DIM_HEAD
                    nc.vector.tensor_scalar_mul(o_q[:, po:po + DIM_HEAD],
                                                oq[:, s, 0:DIM_HEAD],
                                                rec[:, 0:1])
                ps = proj_ps.tile([P, 512], f32, name="ps_proj", tag="proj")
                nc.tensor.transpose(ps[:, 0:P], o_q[:], ident[:])
                nc.vector.tensor_copy(out=o_sb[:, ic, s * P:(s + 1) * P],
                                      in_=ps[:, 0:P])
                if do_m5:
                    emit_m5_final(s)

            # ---- attention: head pairs (even head rows 0-63, odd 64-127) --
            # Warm up the PE pstate (it ramps to full clock after ~3us of
            # continuous work) on throwaway matmuls while the first input
            # DMAs are still in flight.
            warm = big.tile([P, 512], bf, name="warm")
            nc.vector.memset(warm[:], 0.5)
            wps = proj_ps.tile([P, 512], f32, name="ps_proj", tag="proj")
            for _ in range(4):
                nc.tensor.matmul(wps, warm[:, 0:P], warm[:],
                                 start=True, stop=True)
            # Prologue: the first exp needs only k chunk 0 (ctx cols 0:128)
            # and q (ic=0); emit that k sliver as its own accumulation
            # group so the first scores fire as soon as ~640KB has landed.
            ps0 = proj_ps.tile([P, 512], f32, name="ps_proj", tag="proj")
            for kc in range(DC):
                nc.tensor.matmul(ps0[:, 0:P], wk_sb[:, kc, 0:P],
                                 ctx_sb[:, kc, 0:P],
                                 start=(kc == 0), stop=(kc == DC - 1))
            nc.vector.tensor_copy(out=k_sb[:, 0, 0:P], in_=ps0[:, 0:P])
            ps1 = proj_ps.tile([P, 512], f32, name="ps_proj", tag="proj")
            for kc in range(DC):
                nc.tensor.matmul(ps1[:, 0:384], wk_sb[:, kc, 0:P],
                                 ctx_sb[:, kc, P:512],
                                 start=(kc == 0), stop=(kc == DC - 1))
            nc.vector.tensor_copy(out=k_sb[:, 0, P:512], in_=ps1[:, 0:384])
            emit_m0(0)
            prev_drain = None     # (hp, (oq0, oq1)) awaiting normalize
            for hp in range(HEADS // 2):
                ic = hp
                oq0 = oq_ps.tile([P, NQS, DIM_HEAD + 1], f32, name="oq0",
                                 tag="oq0")
                oq1 = oq_ps.tile([P, NQS, DIM_HEAD + 1], f32, name="oq1",
                                 tag="oq1")
                # A PSUM bank holds only ONE open accumulation group at a
                # time; interleaved start-flagged groups in a bank silently
                # reset each other.  Instead: zero the banks, then
                # accumulate-only matmuls (start=False) which do per-address
                # accumulation irrespective of group state.  The memsets are
                # emitted only after the previous head pair's drains.
                if prev_drain is None:
                    nc.vector.memset(oq0[:], 0.0)
                    nc.vector.memset(oq1[:], 0.0)
                pend = []
                last = hp == HEADS // 2 - 1

                def emit_av(j, pT):
                    for par, oq in ((0, oq0), (1, oq1)):
                        for s in range(NQS):
                            nc.tensor.matmul(
                                oq[:, s, :],
                                pT[:, par, s * P:(s + 1) * P],
                                v_sb[:, j, 2 * hp + par, :],
                                start=False, stop=(j == NCHUNK - 1),
                                skip_group_check=True)

                for j in range(NCHUNK):
                    # Projection bursts due this iteration.  Each head pair
                    # computes its OWN k ladder one nt-block ahead of the
                    # scores that use it, plus the NEXT pair's first block
                    # mid-loop -- this spreads K-proj evenly over all four
                    # pairs instead of piling it into hp0.
                    bursts = []
                    if j % 4 == 1 and j // 4 + 1 < NT:
                        bursts.append(("m1", ic, j // 4 + 1))
                    if hp == 0 and j in (2, 6, 10):
                        # q(ic=1..3) early, they gate later head pairs
                        bursts.append(("m0", j // 4 + 1, 0))
                    if hp < HEADS // 2 - 1 and j == 14:
                        bursts.append(("m1", hp + 1, 0))
                    # first half of each burst before the scores, second half
                    # after, so the exp stream is never starved by a long
                    # PE burst
                    pre = []
                    for kind, a, b in bursts:
                        if kind == "m1":
                            pre.append((kind, a, b, m1_first(a, b)))
                        else:
                            pre.append((kind, a, b, m0_first(a)))

                    # High priority: the exp stream paces the whole kernel,
                    # so the scheduler should issue a score pair the moment
                    # its sc buffer frees (the 2-deep sc pool caps run-ahead)
                    sc = score_ps.tile([P, 2, 512], f32, name="sc", tag="sc")
                    with tc.high_priority(offset=64):
                        nc.tensor.matmul(
                            sc[:, 0, :nq],
                            k_sb[0:DIM_HEAD, ic, j * P:(j + 1) * P],
                            q_sb[0:DIM_HEAD, ic, :], start=True, stop=True)
                        nc.tensor.matmul(
                            sc[:, 1, :nq],
                            k_sb[DIM_HEAD:P, ic, j * P:(j + 1) * P],
                            q_sb[DIM_HEAD:P, ic, :], start=True, stop=True)
                    pT = pTp.tile([P, 2, 512], bf, name="pT", tag="pT")
                    if need_mask:
                        nc.scalar.activation(
                            pT[:, :, :nq], sc[:, :, :nq], Exp,
                            bias=mb_sb[:, j, None], scale=0.125)
                    else:
                        nc.scalar.activation(
                            pT[:, :, :nq], sc[:, :, :nq], Exp, scale=0.125)
                    pend.append((j, pT))

                    for kind, a, b, ps in pre:
                        if kind == "m1":
                            m1_rest(a, b, ps)
                        else:
                            m0_rest(a, ps)
                    emit_v(j, hp)
                    if prev_drain is not None and j <= NQS - 1:
                        emit_drain(prev_drain[0], prev_drain[1], j, False)
                        if j == NQS - 1:
                            prev_drain = None
                            nc.vector.memset(oq0[:], 0.0)
                            nc.vector.memset(oq1[:], 0.0)
                    if PARTIAL_M5 and hp >= 1 and NQS <= j < 2 * NQS:
                        emit_m5_partial(hp - 1, j - NQS)
                    skew = SKEW if (not last or j < 2 * NQS) else 2
                    while len(pend) > skew:
                        emit_av(*pend.pop(0))
                for item in pend:
                    emit_av(*item)
                prev_drain = (hp, (oq0, oq1))

            # last head pair: drain + out-projection interleaved per q-slice
            for s in range(NQS):
                emit_drain(prev_drain[0], prev_drain[1], s, True)

    nc.compile()
    return nc


def _get_program(need_mask: bool):
    if need_mask not in _PROGRAMS:
        _PROGRAMS[need_mask] = _build(need_mask)
    return _PROGRAMS[need_mask]


def _prep_inputs(x, context, mask, Wq, Wkv, Wo, bo):
    """Host-side shard + transpose + cast. Returns list of per-core in_maps."""
    x = np.asarray(x, dtype=np.float32)
    context = np.asarray(context, dtype=np.float32)
    mask = np.asarray(mask)
    Wq = np.asarray(Wq, dtype=np.float32)
    Wkv = np.asarray(Wkv, dtype=np.float32)
    Wo = np.asarray(Wo, dtype=np.float32)
    bo = np.asarray(bo, dtype=np.float32)

    need_mask = not bool(mask.all())
    wqT = np.ascontiguousarray(Wq.T).astype(BF16)
    wkT = np.ascontiguousarray(Wkv[:D].T).astype(BF16)
    wvT = np.ascontiguousarray(Wkv[D:].T).astype(BF16)
    woT = np.ascontiguousarray(Wo.T).astype(BF16)
    bo2 = bo.reshape(1, D)

    ctxTs = [np.ascontiguousarray(context[b].T).astype(BF16)
             for b in range(B)]
    if need_mask:
        # additive pre-exp bias: 0 where visible, -1e30 where masked
        mb = [np.where(mask[b], 0.0, -1e30).astype(np.float32)
              .reshape(NCHUNK, P).T.copy() for b in range(B)]

    in_maps = []
    for c in range(N_CORES):
        b, qh = divmod(c, 2)
        qs = qh * NQ_SH
        m = {
            "xT": np.ascontiguousarray(x[b, qs:qs + NQ_SH, :].T).astype(BF16),
            "ctxT": ctxTs[b],
            "wqT": wqT, "wkT": wkT, "wvT": wvT, "woT": woT,
            "bo": bo2,
        }
        if need_mask:
            m["maskb"] = mb[b]
        in_maps.append(m)
    return in_maps, need_mask


def run_sharded(inputs, trace=False):
    """Run on 8 cores; returns (full_output, BassKernelResults)."""
    from concourse import bass_utils
    in_maps, need_mask = _prep_inputs(**inputs)
    nc = _get_program(need_mask)
    res = bass_utils.run_bass_kernel_spmd(
        nc, in_maps, core_ids=list(range(N_CORES)), trace=trace)
    out = np.empty((B, NQ, D), dtype=np.float32)
    for c in range(N_CORES):
        b, qh = divmod(c, 2)
        qs = qh * NQ_SH
        out[b, qs:qs + NQ_SH, :] = res.results[c]["y"]
    return out, res


def kernel(**inputs) -> np.ndarray:
    out, _ = run_sharded(inputs, trace=False)
    return out
